# revision 1
# baseline (speedup 1.0000x reference)
"""Trainium2 Bass kernel for nn_Attention_38130719654002 (sparse_attention).

Strategy
--------
The reference builds a huge [B,H,T,T,2d] weighted_kv tensor (135 MB) and runs a
Conv2d(256->256, k3, s2) over B*T=514 images assembled from it, followed by a
tiny 65-key attention per (b,h,t). 97% of all FLOPs (19.4 GMAC) live in that
conv. We express the conv as ONE dense matmul via im2col:

    co[pix, o] = X[pix, (c',dy,dx)] @ W[(c',dy,dx), o]        K=2304, M=256

X rows (= conv-input patches) are built host-side directly from the rank-1
structure weighted[b,h,t1,:,:] = score ⊗ kv (cheap numpy index gymnastics, no
FLOPs of substance). Pixels (514*64 = 32,896 rows, padded to 33,280) are
sharded 8 ways across NeuronCores; weights are replicated. Each core runs an
18-Ktile x 2-Mtile x 9-Nchunk bf16 matmul with f32 PSUM accumulation.
The 65-key attention tail (8.5 MMAC) runs host-side.
"""

import math
import sys

import numpy as np

sys.path.insert(0, "/opt/trn_rl_repo")
sys.path.insert(0, "/opt/pypackages")

import ml_dtypes  # noqa: E402

import concourse.bass as bass  # noqa: E402
import concourse.mybir as mybir  # noqa: E402
import concourse.tile as tile  # noqa: E402
from concourse import bacc  # noqa: E402
from concourse.bass_utils import run_bass_kernel_spmd  # noqa: E402

B, T, C, H = 2, 257, 128, 8
D = C // H            # 16
HH = WW = 16          # spatial
EPS = 1e-5
N_CORES = 8
K_DIM = 2 * C * 9     # 2304 im2col columns
N_IMG = B * T         # 514
PIX = N_IMG * 64      # 32896 output pixels
PIX_PAD = 33280       # 8 * 4160
PIX_CORE = PIX_PAD // N_CORES  # 4160

_CACHED = {}


def _build_graph():
    """Per-core SPMD graph: out[256, 4160] = wt[2304, 256].T @ xt[2304, 4160]."""
    if "nc" in _CACHED:
        return _CACHED["nc"]
    nc = bacc.Bacc("TRN2", target_bir_lowering=False)
    xt = nc.declare_dram_parameter("xt", [K_DIM, PIX_CORE], mybir.dt.bfloat16,
                                   isOutput=False)
    wt = nc.declare_dram_parameter("wt", [K_DIM, 256], mybir.dt.bfloat16,
                                   isOutput=False)
    out = nc.declare_dram_parameter("out", [256, PIX_CORE], mybir.dt.float32,
                                    isOutput=True)

    KT = K_DIM // 128  # 18 K tiles
    chunks = [(i * 512, 512) for i in range(8)] + [(4096, 64)]
    xt_r = xt.rearrange("(kt p) n -> p kt n", p=128)   # [128, 18, 4160]
    wt_r = wt.rearrange("(kt p) m -> p kt m", p=128)   # [128, 18, 256]

    with tile.TileContext(nc) as tc:
        with (
            tc.tile_pool(name="wpool", bufs=1) as wpool,
            tc.tile_pool(name="xpool", bufs=1) as xpool,
            tc.tile_pool(name="opool", bufs=18) as opool,
            tc.tile_pool(name="psum", bufs=8, space=bass.MemorySpace.PSUM) as pp,
        ):
            w_sb = wpool.tile([128, KT, 256], mybir.dt.bfloat16)
            nc.gpsimd.dma_start(w_sb[:], wt_r[:])
            x_sb = xpool.tile([128, KT, PIX_CORE], mybir.dt.bfloat16)
            for ci_, (c0, cw) in enumerate(chunks):
                if ci_ == 0:
                    # split the first chunk per K-tile so matmul kk=0 can
                    # start after ~130KB of DMA instead of the full 2.4MB
                    for kk in range(KT):
                        nc.gpsimd.dma_start(x_sb[:, kk, c0:c0 + cw],
                                            xt_r[:, kk, c0:c0 + cw])
                else:
                    nc.gpsimd.dma_start(x_sb[:, :, c0:c0 + cw],
                                        xt_r[:, :, c0:c0 + cw])
            for c0, cw in chunks:
                for m in range(2):
                    acc = pp.tile([128, 512], mybir.dt.float32, tag="acc")
                    for kk in range(KT):
                        nc.tensor.matmul(
                            acc[:, :cw],
                            w_sb[:, kk, m * 128:(m + 1) * 128],
                            x_sb[:, kk, c0:c0 + cw],
                            start=(kk == 0), stop=(kk == KT - 1))
                    o_sb = opool.tile([128, 512], mybir.dt.float32, tag="o")
                    nc.vector.tensor_copy(o_sb[:, :cw], acc[:, :cw])
                    nc.gpsimd.dma_start(out[m * 128:(m + 1) * 128, c0:c0 + cw],
                                        o_sb[:, :cw])
    nc.compile()
    _CACHED["nc"] = nc
    return nc


def _softmax(x, axis=-1):
    m = np.max(x, axis=axis, keepdims=True)
    e = np.exp(x - m)
    return e / np.sum(e, axis=axis, keepdims=True)


def _erf(x):
    try:
        from scipy.special import erf
        return erf(x)
    except Exception:
        return np.vectorize(math.erf)(x).astype(x.dtype)


def kernel(x, attn_score_grad, dwq_w, dwk_w, dwv_w, bnq_g, bnq_b, bnk_g, bnk_b,
           bnv_g, bnv_b, Wq, Wk, Wv, conv_w, conv_b, bn2_g, bn2_b, h, w,
           _timing=None):
    x = np.asarray(x, np.float32)
    asg = np.asarray(attn_score_grad, np.float32)
    s_bn = np.float32(1.0 / math.sqrt(1.0 + EPS))

    # ---- host: q/k/v conv projections + linear projections (tiny) ----
    cls = x[:, :1]                                            # [B,1,C]
    xs = x[:, 1:].reshape(B, HH, WW, C).transpose(0, 3, 1, 2)  # [B,C,16,16]
    xp = np.pad(xs, ((0, 0), (0, 0), (1, 1), (1, 1)))

    def conv_proj(dwgt, g, b):
        o = np.zeros_like(xs)
        for dy in range(3):
            for dx in range(3):
                o += xp[:, :, dy:dy + HH, dx:dx + WW] * \
                    dwgt[None, :, 0, dy, dx, None, None]
        o = o * (g * s_bn)[None, :, None, None] + b[None, :, None, None]
        return o.transpose(0, 2, 3, 1).reshape(B, HH * WW, C)

    q = np.concatenate([cls, conv_proj(dwq_w, bnq_g, bnq_b)], 1) @ Wq.T
    k = np.concatenate([cls, conv_proj(dwk_w, bnk_g, bnk_b)], 1) @ Wk.T
    v = np.concatenate([cls, conv_proj(dwv_w, bnv_g, bnv_b)], 1) @ Wv.T
    qh = q.reshape(B, T, H, D).transpose(0, 2, 1, 3)          # [B,H,T,16]
    kh = k.reshape(B, T, H, D).transpose(0, 2, 1, 3)
    vh = v.reshape(B, T, H, D).transpose(0, 2, 1, 3)
    kv = np.concatenate([kh, vh], -1)                         # [B,H,T,32]

    # ---- host: score normalization ----
    first = asg[..., :1]
    rem = asg[..., 1:]
    pos = _softmax(rem / 0.5)
    neg = _softmax(-rem / 0.5)
    score = np.concatenate([first, 0.7 * pos + 0.3 * (1.0 - neg)], -1)

    # ---- host: weighted_kv -> conv-input images -> im2col (index work) ----
    weighted = score[..., None] * kv[:, :, :, None, :]        # [B,H,T,T,32]
    cls_tok = weighted[:, :, :, :1, :].copy()                 # [B,H,T,1,32]
    feat = weighted[:, :, :, 1:, :].reshape(B, T, HH, WW, 2 * C)
    ci = feat.transpose(0, 1, 4, 2, 3).reshape(N_IMG, 2 * C, HH, WW)
    del weighted, feat
    cip = np.pad(ci, ((0, 0), (0, 0), (1, 1), (1, 1)))
    win = np.lib.stride_tricks.sliding_window_view(
        cip, (3, 3), axis=(2, 3))[:, :, ::2, ::2]             # [514,256,8,8,3,3]
    X = win.transpose(0, 2, 3, 1, 4, 5).reshape(PIX, K_DIM)   # rows=(n,oy,ox)
    del ci, cip, win

    s2 = (bn2_g * s_bn).astype(np.float32)
    W_eff = conv_w.reshape(256, K_DIM) * s2[:, None]
    bias_eff = (conv_b * s2 + bn2_b).astype(np.float32)

    Xt = np.zeros((K_DIM, PIX_PAD), dtype=ml_dtypes.bfloat16)
    Xt[:, :PIX] = X.T.astype(ml_dtypes.bfloat16)
    Wt = np.ascontiguousarray(W_eff.T).astype(ml_dtypes.bfloat16)

    # ---- device: sharded big matmul ----
    nc = _build_graph()
    in_maps = [
        {"xt": np.ascontiguousarray(Xt[:, i * PIX_CORE:(i + 1) * PIX_CORE]),
         "wt": Wt}
        for i in range(N_CORES)
    ]
    kw = {}
    if _timing is not None and _timing.get("trace"):
        kw = {"trace": True}
    res = run_bass_kernel_spmd(nc, in_maps, core_ids=list(range(N_CORES)), **kw)
    if _timing is not None:
        _timing["exec_time_ns"] = res.exec_time_ns
        _timing["in_maps"] = in_maps
    co = np.concatenate([r["out"] for r in res.results], axis=1)[:, :PIX]

    # ---- host: bias + attention tail ----
    co = co.astype(np.float32) + bias_eff[:, None]            # [256, PIX]
    co = co.T.reshape(N_IMG, 8, 8, 256).transpose(0, 3, 1, 2)  # [514,256,8,8]
    co = co.reshape(B, T, H, 2 * D, 8, 8).transpose(0, 2, 1, 3, 4, 5)
    cf = co.reshape(B, H, T, 64, 2 * D)
    kvps = np.concatenate([cls_tok, cf], axis=-2)             # [B,H,T,65,32]
    k_ps = kvps[..., :D]
    v_ps = kvps[..., D:]
    logits = np.einsum('bhtd,bhtkd->bhtk', qh, k_ps) * np.float32(C ** -0.5)
    attn = _softmax(logits)
    o = np.einsum('bhtk,bhtkd->bhtd', attn, v_ps)
    o = o.transpose(0, 2, 1, 3).reshape(B, T, C).astype(np.float32)
    return (0.5 * o * (1.0 + _erf(o / np.float32(math.sqrt(2.0))))
            ).astype(np.float32)

